# revision 30
# baseline (speedup 1.0000x reference)
"""BitNet transformer block on 8 Trainium2 NeuronCores.

Sequence-parallel: each core owns 512 of the 4096 (B*T) token rows — two
256-row blocks of one batch, zig-zag paired (blocks c and 7-c) for causal
load balance. Weight slices arrive PRE-TRANSPOSED from the host (each
core gets a 1/8 contraction-dim slice of every W^T), so ternary
quantization is purely elementwise on device and the bf16 ternary
transposed weights AllGather with no on-device transposes.

Attention computes scores TRANSPOSED (kv on partitions, q on free dim):
S^T tiles come straight out of the PE in the layout the PV matmul needs
as lhsT, eliminating all probability transposes. Softmax max is a
pairwise tensor_tensor max tree + gpsimd partition_all_reduce; exp runs
on the activation engine; the softmax denominator accumulates via
ones-vector matmuls in PSUM. Probabilities and V stay fp32 (f32r
matmuls at full PE rate, free dim >= 256) so the attention output
matches the f32 reference closely; o^T returns to token-major via
hi/lo-split bf16 xbar transposes (exact fp32 reconstruction).

DMAs are spread across the three descriptor rings (sync/SP for big
weight streams, scalar/ACT for gathers + transposes, gpsimd/SWDGE for
weight-slice loads + collectives). PSUM: 8 banks as tags ps(5) + oT(2)
+ se(1). Elementwise work alternates between the DVE and Pool engines.

SPMD: all cores run one identical graph; causal geometry is data-driven
via an additive mask input in the transposed layout.
"""

import numpy as np
from contextlib import ExitStack

B, T, D, H, HD, F = 2, 2048, 2048, 16, 128, 8192
NCORES = 8
TPC = 512            # tokens per core
MAGIC = 12582912.0   # 1.5*2**23: +M then -M rounds f32 to nearest-even int
INV127 = float(np.float32(1.0) / np.float32(127.0))
SMSCALE = float(np.float32(1.0) / np.sqrt(np.float32(HD)))
NEG = -1.0e30


def _kv_src(j):
    """(group_rank, local_row) holding kv block j in a gathered buffer."""
    return (j, 0) if j <= 3 else (7 - j, 256)


def build_graph():
    import concourse.bass as bass
    import concourse.bacc as bacc
    import concourse.tile as tile
    from concourse import mybir, bass_isa

    f32 = mybir.dt.float32
    bf16 = mybir.dt.bfloat16
    f32r = mybir.dt.float32r
    Alu = mybir.AluOpType
    Act = mybir.ActivationFunctionType
    Ax = mybir.AxisListType
    Rop = bass_isa.ReduceOp

    nc = bacc.Bacc("TRN2", target_bir_lowering=False, debug=False,
                   num_devices=NCORES)

    x_d = nc.dram_tensor("x", [TPC, D], f32, kind="ExternalInput").ap()
    # pre-transposed weight slices: rows = contraction dim (128-blocks)
    w_d = {
        "wq": nc.dram_tensor("wq_sl", [256, D], f32, kind="ExternalInput").ap(),
        "wk": nc.dram_tensor("wk_sl", [256, D], f32, kind="ExternalInput").ap(),
        "wv": nc.dram_tensor("wv_sl", [256, D], f32, kind="ExternalInput").ap(),
        "wo": nc.dram_tensor("wo_sl", [256, D], f32, kind="ExternalInput").ap(),
        "wu": nc.dram_tensor("wu_sl", [256, F], f32, kind="ExternalInput").ap(),
        "wd": nc.dram_tensor("wd_sl", [1024, D], f32, kind="ExternalInput").ap(),
    }
    mask_d = nc.dram_tensor("maskT", [16, 128, 256], f32,
                            kind="ExternalInput").ap()
    out_d = nc.dram_tensor("out", [TPC, D], f32, kind="ExternalOutput").ap()

    r_all = [list(range(NCORES))]
    r_grp = [[0, 1, 2, 3], [4, 5, 6, 7]]

    # (name, n 128-row blocks, row width)
    worder = [("wk", 2, D), ("wq", 2, D), ("wv", 2, D), ("wo", 2, D),
              ("wu", 2, F), ("wd", 8, D)]
    wmean = {"wq": D * D, "wk": D * D, "wv": D * D, "wo": D * D,
             "wu": F * D, "wd": D * F}
    widx = {nm: i for i, (nm, _, _) in enumerate(worder)}

    with tile.TileContext(nc) as tc, ExitStack() as g:
        dram = g.enter_context(tc.tile_pool(name="dram", bufs=1,
                                            space="DRAM"))
        p0 = g.enter_context(tc.tile_pool(name="p0", bufs=1))
        stats = g.enter_context(tc.tile_pool(name="stats", bufs=4))
        scrA = g.enter_context(tc.tile_pool(name="scrA", bufs=1))
        xqtp = g.enter_context(tc.tile_pool(name="xqtp", bufs=1))
        psum = g.enter_context(tc.tile_pool(name="psum", bufs=5,
                                            space="PSUM"))

        # alternate elementwise work between DVE and Pool engines
        _alt = [0]

        def VE():
            _alt[0] ^= 1
            return nc.vector if _alt[0] else nc.gpsimd

        # ---- DRAM buffers ----
        ar_in = dram.tile([1, 8], f32, name="ar_in")
        ar_out = dram.tile([1, 8], f32, name="ar_out", addr_space="Shared")
        rws_row = dram.tile([1, 8], f32, name="rws_row")
        scl_dram = dram.tile([1, TPC], f32, name="scl_dram")
        rse_dram = dram.tile([16, TPC], f32, name="rse_dram")
        ag_in, ag_out = {}, {}
        for nm, nb, wd_ in worder:
            ag_in[nm] = dram.tile([nb * 128, wd_], bf16, name=f"agi_{nm}")
            ag_out[nm] = dram.tile([nb * 128 * NCORES, wd_], bf16,
                                   name=f"ago_{nm}", addr_space="Shared")
        agk_in = dram.tile([D + 1, TPC], f32, name="agk_in")
        agk_out = dram.tile([4 * (D + 1), TPC], f32, name="agk_out")
        agv_in = dram.tile([TPC, D], bf16, name="agv_in")
        agv_out = dram.tile([4 * TPC, D], bf16, name="agv_out")

        # ---- global SBUF ----
        eps_t = p0.tile([128, 1], f32, name="eps")
        nc.vector.memset(eps_t, 1.0e-5)
        ones_t = p0.tile([128, 1], bf16, name="ones_t")
        nc.vector.memset(ones_t, 1.0)
        rbc = p0.tile([128, 8], f32, name="rbc")
        cqT = p0.tile([128, TPC], f32, name="cqT")
        scaleA = [p0.tile([128, 1], f32, name=f"sA{i}") for i in range(4)]
        scaleB = [p0.tile([128, 1], f32, name=f"sB{i}") for i in range(4)]
        scaleC = [p0.tile([128, 1], f32, name=f"sC{i}") for i in range(4)]
        scaleD = [p0.tile([128, 1], f32, name=f"sD{i}") for i in range(4)]
        rseq = [p0.tile([128, 16], f32, name=f"rseq{i}") for i in range(4)]
        amaxD = [p0.tile([128, 16], f32, name=f"amx{i}") for i in range(4)]
        ou = [p0.tile([128, D], f32, name=f"ou{i}") for i in range(4)]
        xqt = [xqtp.tile([128, TPC], bf16, tag=f"xqt{i}", name=f"xqt{i}")
               for i in range(16)]

        def ln(src, dst):
            st = stats.tile([128, 4, 6], f32, tag="bn", name="bn")
            for sg in range(4):
                nc.vector.bn_stats(out=st[:, sg, :],
                                   in_=src[:, sg * 512:(sg + 1) * 512])
            mv = stats.tile([128, 2], f32, tag="mv", name="mv")
            nc.vector.bn_aggr(out=mv, in_=st)
            sq = stats.tile([128, 1], f32, tag="sq", name="sq")
            nc.scalar.activation(out=sq, in_=mv[:, 1:2], func=Act.Sqrt,
                                 bias=eps_t, scale=1.0)
            rstd = stats.tile([128, 1], f32, tag="rstd", name="rstd")
            nc.vector.reciprocal(out=rstd, in_=sq)
            nc.vector.tensor_scalar(out=dst, in0=src, scalar1=mv[:, 0:1],
                                    scalar2=rstd, op0=Alu.subtract,
                                    op1=Alu.mult)

        def quant_rq(amax, scale_out):
            nc.vector.tensor_scalar(out=scale_out, in0=amax, scalar1=INV127,
                                    scalar2=None, op0=Alu.mult)
            rq = stats.tile([128, 1], f32, tag="rq", name="rq")
            nc.vector.tensor_scalar(out=rq, in0=scale_out, scalar1=1e-8,
                                    scalar2=None, op0=Alu.add)
            nc.vector.reciprocal(out=rq, in_=rq)
            return rq

        def quant(h, xq_out, scale_out):
            # NOTE: destroys h (pass 1 is in-place)
            amax = stats.tile([128, 1], f32, tag="amax", name="amax")
            nc.vector.tensor_reduce(out=amax, in_=h, axis=Ax.X, op=Alu.max,
                                    apply_absolute_value=True)
            rq = quant_rq(amax, scale_out)
            nc.vector.tensor_scalar(out=h, in0=h, scalar1=rq, scalar2=MAGIC,
                                    op0=Alu.mult, op1=Alu.add)
            nc.vector.tensor_scalar(out=xq_out, in0=h, scalar1=MAGIC,
                                    scalar2=None, op0=Alu.subtract)

        # =============================================================
        # P0: weight abs-sums + LN1/quantA overlap, then ternary quant
        # =============================================================
        with ExitStack() as s1:
            wraw = s1.enter_context(tc.tile_pool(name="wraw", bufs=3))
            wqnt = s1.enter_context(tc.tile_pool(name="wqnt", bufs=2))
            xqap = s1.enter_context(tc.tile_pool(name="xqap", bufs=1))

            # ---- stats pass: 1MB chunk loads, abs-sum reduce ----
            pp = p0.tile([128, 24], f32, name="pp")
            pi = 0
            pslices = {}
            for nm, nb, wd_ in worder:
                p_start = pi
                for b in range(nb):
                    for ck in range(wd_ // 2048):
                        t = wraw.tile([128, 2048], f32, tag="wraw",
                                      name="wraw")
                        nc.gpsimd.dma_start(
                            out=t,
                            in_=w_d[nm][b * 128:(b + 1) * 128,
                                        ck * 2048:(ck + 1) * 2048])
                        nc.vector.tensor_reduce(
                            out=pp[:, pi:pi + 1], in_=t, axis=Ax.X,
                            op=Alu.add, apply_absolute_value=True)
                        pi += 1
                pslices[nm] = (p_start, pi)
            wsum = p0.tile([128, 6], f32, name="wsum")
            for nm, nb, wd_ in worder:
                a0, a1 = pslices[nm]
                nc.vector.tensor_reduce(out=wsum[:, widx[nm]:widx[nm] + 1],
                                        in_=pp[:, a0:a1], axis=Ax.X,
                                        op=Alu.add)
            wred = p0.tile([128, 6], f32, name="wred")
            nc.gpsimd.partition_all_reduce(out_ap=wred, in_ap=wsum,
                                           channels=128, reduce_op=Rop.add)
            ws_row = p0.tile([1, 8], f32, name="ws_row")
            nc.vector.memset(ws_row, 0.0)
            nc.vector.tensor_copy(out=ws_row[0:1, 0:6], in_=wred[0:1, :])
            nc.gpsimd.dma_start(out=ar_in[:], in_=ws_row[:])
            nc.gpsimd.collective_compute(
                "AllReduce", Alu.add, replica_groups=r_all,
                ins=[ar_in[:].opt()], outs=[ar_out[:].opt()])

            # ---- LN1 + token quant (overlaps AllReduce latency) ----
            xqa = [xqap.tile([128, D], bf16, name=f"xqa{i}")
                   for i in range(4)]
            for tt in range(4):
                xt = scrA.tile([128, D], f32, tag="work", name="work")
                nc.sync.dma_start(out=xt,
                                  in_=x_d[tt * 128:(tt + 1) * 128, :])
                ln(xt, xt)
                quant(xt, xqa[tt], scaleA[tt])
            for dt in range(16):
                for tt in range(4):
                    nc.scalar.dma_start(
                        out=xqt[dt][:, tt * 128:(tt + 1) * 128],
                        in_=xqa[tt][:, dt * 128:(dt + 1) * 128],
                        transpose=True)
            # cqT: per-q scale row broadcast to all partitions, * 1/sqrt(hd)
            for tt in range(4):
                nc.scalar.dma_start(
                    out=scl_dram[0:1, tt * 128:(tt + 1) * 128],
                    in_=scaleA[tt])
            nc.sync.dma_start(out=cqT,
                              in_=scl_dram[:].partition_broadcast(128))
            nc.vector.tensor_scalar(out=cqT, in0=cqT, scalar1=SMSCALE,
                                    scalar2=None, op0=Alu.mult)

            # ---- finish the absmean scales ----
            rrow = stats.tile([1, 8], f32, tag="rrow", name="rrow")
            nc.sync.dma_start(out=rrow, in_=ar_out[:])
            for nm, nb, wd_ in worder:
                i = widx[nm]
                nc.vector.tensor_scalar(out=rrow[0:1, i:i + 1],
                                        in0=rrow[0:1, i:i + 1],
                                        scalar1=1.0 / wmean[nm], scalar2=1e-8,
                                        op0=Alu.mult, op1=Alu.add)
            nc.vector.reciprocal(out=rrow, in_=rrow)
            nc.gpsimd.dma_start(out=rws_row[:], in_=rrow[:])
            nc.sync.dma_start(out=rbc,
                              in_=rws_row[:].partition_broadcast(128))

            # ---- ternary quant (elementwise, pre-transposed layout) ----
            for nm, nb, wd_ in worder:
                i = widx[nm]
                for b in range(nb):
                    for ck in range(wd_ // 2048):
                        t = wraw.tile([128, 2048], f32, tag="wraw",
                                      name="wraw")
                        nc.gpsimd.dma_start(
                            out=t,
                            in_=w_d[nm][b * 128:(b + 1) * 128,
                                        ck * 2048:(ck + 1) * 2048])
                        e = VE()
                        e.tensor_scalar(out=t, in0=t,
                                        scalar1=rbc[:, i:i + 1],
                                        scalar2=MAGIC,
                                        op0=Alu.mult, op1=Alu.add)
                        e.tensor_scalar(out=t, in0=t, scalar1=MAGIC,
                                        scalar2=None, op0=Alu.subtract)
                        q3 = wqnt.tile([128, 2048], bf16, tag="wq3",
                                       name="wq3")
                        e.tensor_scalar(out=q3, in0=t,
                                        scalar1=-1.0, scalar2=1.0,
                                        op0=Alu.max, op1=Alu.min)
                        nc.gpsimd.dma_start(
                            out=ag_in[nm][b * 128:(b + 1) * 128,
                                          ck * 2048:(ck + 1) * 2048],
                            in_=q3)
                nc.gpsimd.collective_compute(
                    "AllGather", Alu.bypass, replica_groups=r_all,
                    ins=[ag_in[nm][:].opt()], outs=[ag_out[nm][:].opt()])

        def wT(nm, dt, c0, c1):
            """[128, c1-c0] lhsT chunk: rows = contraction dims
            dt*128..dt*128+128, cols = output dims c0..c1."""
            d0 = dt * 128
            per = 1024 if nm == "wd" else 256
            rk, off = d0 // per, d0 % per
            return ag_out[nm][rk * per + off:rk * per + off + 128, c0:c1]

        # =============================================================
        # P1: K, Q, V projections (K first so its AllGather starts early)
        # =============================================================
        s12 = g.enter_context(ExitStack())
        qscp = s12.enter_context(tc.tile_pool(name="qscp", bufs=1))
        qsc = [qscp.tile([128, TPC], f32r, name=f"qsc{hh}")
               for hh in range(16)]
        with ExitStack() as s2:
            wsA = s2.enter_context(tc.tile_pool(name="wsA", bufs=3))
            kvp = s2.enter_context(tc.tile_pool(name="kvp", bufs=2))

            for nm in ("wk", "wq"):
                for grp in range(4):
                    pss = [psum.tile([128, TPC], f32, tag="ps", name="ps")
                           for _ in range(4)]
                    for dt in range(16):
                        wt = wsA.tile([128, 512], bf16, tag="wstk",
                                      name="wstk", bufs=4)
                        nc.sync.dma_start(
                            out=wt,
                            in_=wT(nm, dt, grp * 512, (grp + 1) * 512))
                        for hh in range(4):
                            nc.tensor.matmul(
                                pss[hh][:],
                                wt[:, hh * 128:(hh + 1) * 128],
                                xqt[dt][:], start=(dt == 0),
                                stop=(dt == 15))
                    for hh in range(4):
                        ha = grp * 4 + hh
                        if nm == "wq":
                            nc.vector.tensor_tensor(
                                out=qsc[ha], in0=pss[hh][:], in1=cqT,
                                op=Alu.mult)
                        else:
                            kd = kvp.tile([128, TPC], f32, tag="kdr",
                                          name="kdr")
                            nc.scalar.activation(out=kd, in_=pss[hh][:],
                                                 func=Act.Copy)
                            nc.scalar.dma_start(
                                out=agk_in[ha * 128:(ha + 1) * 128, :],
                                in_=kd)
                if nm == "wk":
                    for tt in range(4):
                        nc.scalar.dma_start(
                            out=agk_in[D:D + 1, tt * 128:(tt + 1) * 128],
                            in_=scaleA[tt])
                    nc.gpsimd.collective_compute(
                        "AllGather", Alu.bypass, replica_groups=r_grp,
                        ins=[agk_in[:].opt()], outs=[agk_out[:].opt()])

            for tp_ in range(4):
                pss = [psum.tile([128, TPC], f32, tag="ps", name="ps")
                       for _ in range(4)]
                for dt in range(16):
                    wt = wsA.tile([128, 2048], bf16, tag="wstv",
                                  name="wstv", bufs=3)
                    nc.sync.dma_start(out=wt, in_=wT("wv", dt, 0, D))
                    for oc in range(4):
                        nc.tensor.matmul(
                            pss[oc][:],
                            xqt[dt][:, tp_ * 128:(tp_ + 1) * 128],
                            wt[:, oc * 512:(oc + 1) * 512],
                            start=(dt == 0), stop=(dt == 15))
                for oc in range(4):
                    vsb = kvp.tile([128, TPC], bf16, tag="vdr",
                                   name="vdr")
                    nc.scalar.activation(out=vsb, in_=pss[oc][:],
                                         func=Act.Copy,
                                         scale=scaleA[tp_])
                    nc.scalar.dma_start(
                        out=agv_in[tp_ * 128:(tp_ + 1) * 128,
                                   oc * 512:(oc + 1) * 512], in_=vsb)
            nc.gpsimd.collective_compute(
                "AllGather", Alu.bypass, replica_groups=r_grp,
                ins=[agv_in[:].opt()], outs=[agv_out[:].opt()])

        # =============================================================
        # P2: attention, transposed scores (kv on partitions)
        # =============================================================
        with ExitStack() as s3:
            attnp = s3.enter_context(tc.tile_pool(name="attnp", bufs=2))
            ssbp = s3.enter_context(tc.tile_pool(name="ssbp", bufs=2))
            mskp = s3.enter_context(tc.tile_pool(name="mskp", bufs=1))
            scrT = s3.enter_context(tc.tile_pool(name="scrT", bufs=2))

            maskT = []
            for kt in range(16):
                mt = mskp.tile([128, 256], bf16, name=f"mskT{kt}")
                nc.gpsimd.dma_start(out=mt, in_=mask_d[kt])
                maskT.append(mt)
            skv_bc = mskp.tile([128, 2048], f32, name="skv_bc")
            for j in range(8):
                gk, lr = _kv_src(j)
                nc.sync.dma_start(
                    out=skv_bc[:, j * 256:(j + 1) * 256],
                    in_=agk_out[gk * (D + 1) + D:gk * (D + 1) + D + 1,
                                lr:lr + 256].partition_broadcast(128))

            state = {}

            def attn_front(h):
                ksc = attnp.tile([128, 2048], f32, tag="ksc", name="ksc")
                for j in range(8):
                    gk, lr = _kv_src(j)
                    eng = nc.scalar if j < 4 else nc.sync
                    eng.dma_start(
                        out=ksc[:, j * 256:(j + 1) * 256],
                        in_=agk_out[gk * (D + 1) + h * 128:
                                    gk * (D + 1) + (h + 1) * 128,
                                    lr:lr + 256])
                nc.vector.tensor_mul(ksc[:].bitcast(f32r), ksc, skv_bc)
                vts = [attnp.tile([128, 128], bf16, tag=f"vt{kt}",
                                  name=f"vt{kt}") for kt in range(16)]
                for kt in range(16):
                    j, half = kt // 2, kt % 2
                    gk, lr = _kv_src(j)
                    eng = nc.scalar if kt % 2 else nc.sync
                    eng.dma_start(
                        out=vts[kt],
                        in_=agv_out[gk * TPC + lr + half * 128:
                                    gk * TPC + lr + half * 128 + 128,
                                    h * 128:(h + 1) * 128])
                # scores + evict/mask
                ssb = []
                prb = []
                for kt in range(16):
                    w_ = TPC if kt < 8 else 256
                    sb = ssbp.tile([128, w_], f32, tag=f"ssb{kt}",
                                   name=f"ssb{kt}", bufs=1)
                    ssb.append(sb)
                    pb = ssbp.tile([128, w_], bf16, tag=f"prb{kt}",
                                   name=f"prb{kt}")
                    prb.append(pb)
                for kt in range(16):
                    if kt < 8:
                        ps = psum.tile([128, TPC], f32, tag="ps",
                                       name="ps")
                        nc.tensor.matmul(
                            ps[:],
                            ksc[:, kt * 128:(kt + 1) * 128]
                            .bitcast(f32r),
                            qsc[h][:], start=True, stop=True)
                        nc.vector.scalar_tensor_tensor(
                            out=ssb[kt][:, 0:256], in0=ps[:, 0:256],
                            scalar=1.0, in1=maskT[kt],
                            op0=Alu.mult, op1=Alu.add)
                        nc.scalar.activation(
                            out=ssb[kt][:, 256:TPC],
                            in_=ps[:, 256:TPC], func=Act.Copy)
                    else:
                        ps = psum.tile([128, 256], f32, tag="ps",
                                       name="ps")
                        nc.tensor.matmul(
                            ps[:],
                            ksc[:, kt * 128:(kt + 1) * 128]
                            .bitcast(f32r),
                            qsc[h][:, 256:TPC], start=True, stop=True)
                        nc.vector.scalar_tensor_tensor(
                            out=ssb[kt][:], in0=ps[:],
                            scalar=1.0, in1=maskT[kt],
                            op0=Alu.mult, op1=Alu.add)
                # running max per half (in-place chains, one engine each)
                h0 = [ssb[kt][:, 0:256] for kt in range(8)]
                h1 = ([ssb[kt][:, 256:TPC] for kt in range(8)]
                      + [ssb[kt][:] for kt in range(8, 16)])
                allmax = attnp.tile([128, TPC], f32, tag="allmax",
                                    name="allmax", bufs=1)

                def chain(sl, dst, eng):
                    mc = stats.tile([128, 256], f32, tag="mc", name="mc",
                                    bufs=4)
                    eng.tensor_tensor(out=mc, in0=sl[0], in1=sl[1],
                                      op=Alu.max)
                    for s in sl[2:]:
                        eng.tensor_tensor(out=mc, in0=mc, in1=s,
                                          op=Alu.max)
                    nc.gpsimd.partition_all_reduce(
                        out_ap=dst, in_ap=mc, channels=128,
                        reduce_op=Rop.max)

                chain(h0, allmax[:, 0:256], nc.vector)
                chain(h1, allmax[:, 256:TPC], nc.vector)
                # subtract max into staging, exp back into ssb (fp32)
                for kt in range(16):
                    w_ = TPC if kt < 8 else 256
                    sdiff = ssbp.tile([128, w_], f32, tag="sdiff",
                                      name="sdiff", bufs=2)
                    am = allmax if kt < 8 else allmax[:, 256:TPC]
                    nc.vector.tensor_tensor(out=sdiff, in0=ssb[kt],
                                            in1=am, op=Alu.subtract)
                    nc.scalar.activation(out=prb[kt][:], in_=sdiff,
                                         func=Act.Exp)
                state[h] = (prb, vts)

            def attn_back(h):
                prb, vts = state.pop(h)
                # softmax denominators via ones-matmuls
                se_ps = psum.tile([1, TPC], f32, tag="se", name="se_ps",
                                  bufs=1)
                for kt in range(8):
                    nc.tensor.matmul(se_ps[0:1, 0:256],
                                     ones_t[:], prb[kt][:, 0:256],
                                     start=(kt == 0), stop=(kt == 7))
                for kt in range(16):
                    pr = (prb[kt][:, 256:TPC] if kt < 8 else prb[kt][:])
                    nc.tensor.matmul(se_ps[0:1, 256:TPC],
                                     ones_t[:], pr,
                                     start=(kt == 0), stop=(kt == 15))
                sst = scrT.tile([1, TPC], f32, tag="sst", name="sst")
                nc.scalar.activation(out=sst, in_=se_ps[0:1, :],
                                     func=Act.Copy)
                srec = scrT.tile([1, TPC], f32, tag="srec", name="srec")
                nc.vector.reciprocal(out=srec, in_=sst)
                nc.scalar.dma_start(out=rse_dram[h:h + 1, :].opt(),
                                    in_=srec)
                # PV: o^T[hd, q]
                oT = psum.tile([128, TPC], f32, tag="oT", name="oT",
                               bufs=2)
                for kt in range(8):
                    nc.tensor.matmul(oT[:, 0:256],
                                     vts[kt][:], prb[kt][:, 0:256],
                                     start=(kt == 0), stop=(kt == 7))
                for kt in range(16):
                    pr = (prb[kt][:, 256:TPC] if kt < 8 else prb[kt][:])
                    nc.tensor.matmul(oT[:, 256:TPC],
                                     vts[kt][:], pr,
                                     start=(kt == 0), stop=(kt == 15))
                # evict hi/lo bf16, transpose, recombine into ou (fp32)
                obhi = scrT.tile([128, TPC], bf16, tag="obhi", name="obhi")
                nc.scalar.activation(out=obhi, in_=oT[:], func=Act.Copy)
                oblo = scrT.tile([128, TPC], bf16, tag="oblo", name="oblo")
                nc.vector.scalar_tensor_tensor(out=oblo, in0=oT[:],
                                               scalar=1.0, in1=obhi,
                                               op0=Alu.mult,
                                               op1=Alu.subtract)
                for qt in range(4):
                    thi = scrT.tile([128, 128], bf16, tag="thi",
                                    name="thi", bufs=4)
                    nc.scalar.dma_start(
                        out=thi, in_=obhi[:, qt * 128:(qt + 1) * 128],
                        transpose=True)
                    tlo = scrT.tile([128, 128], bf16, tag="tlo",
                                    name="tlo", bufs=4)
                    nc.scalar.dma_start(
                        out=tlo, in_=oblo[:, qt * 128:(qt + 1) * 128],
                        transpose=True)
                    VE().tensor_tensor(
                        out=ou[qt][:, h * 128:(h + 1) * 128],
                        in0=thi, in1=tlo, op=Alu.add)

            for h in range(17):
                if h < 16:
                    attn_front(h)
                if h >= 1:
                    attn_back(h - 1)

            # 1/se back to token-major via strided-AP loads (tiny)
            for tt in range(4):
                nc.scalar.dma_start(
                    out=rseq[tt],
                    in_=rse_dram[:, tt * 128:(tt + 1) * 128]
                    .rearrange("h t -> t h"))

        s12.close()

        # =============================================================
        # P3: out-projection + residual
        # =============================================================
        with ExitStack() as s4:
            x2p = s4.enter_context(tc.tile_pool(name="x2p", bufs=1))
            scrR = s4.enter_context(tc.tile_pool(name="scrR", bufs=2))

            with ExitStack() as s4a:
                oqp = s4a.enter_context(tc.tile_pool(name="oqp", bufs=1))
                oq = [oqp.tile([128, D], bf16, name=f"oq{i}")
                      for i in range(4)]
                for tt in range(4):
                    for h in range(16):
                        VE().tensor_scalar(
                            out=ou[tt][:, h * 128:(h + 1) * 128],
                            in0=ou[tt][:, h * 128:(h + 1) * 128],
                            scalar1=rseq[tt][:, h:h + 1], scalar2=None,
                            op0=Alu.mult)
                    quant(ou[tt], oq[tt], scaleB[tt])
                for dt in range(16):
                    for tt in range(4):
                        nc.scalar.dma_start(
                            out=xqt[dt][:, tt * 128:(tt + 1) * 128],
                            in_=oq[tt][:, dt * 128:(dt + 1) * 128],
                            transpose=True)

            x2 = [x2p.tile([128, D], f32, name=f"x2_{i}")
                  for i in range(4)]
            with ExitStack() as s4b:
                wsP3 = s4b.enter_context(tc.tile_pool(name="wsP3",
                                                      bufs=3))
                for grp in range(2):
                    pss = [psum.tile([128, TPC], f32, tag="ps", name="ps")
                           for _ in range(5)]
                    pss += [psum.tile([128, TPC], f32, tag="oT",
                                      name="psx", bufs=2)
                            for _ in range(2)]
                    pss += [psum.tile([128, TPC], f32, tag="se",
                                      name="psy", bufs=1)]
                    for dt in range(16):
                        wt = wsP3.tile([128, 1024], bf16, tag="wsto",
                                       name="wsto")
                        nc.sync.dma_start(
                            out=wt,
                            in_=wT("wo", dt, grp * 1024, (grp + 1) * 1024))
                        for tt in range(4):
                            for oc in range(2):
                                nc.tensor.matmul(
                                    pss[tt * 2 + oc][:],
                                    xqt[dt][:, tt * 128:(tt + 1) * 128],
                                    wt[:, oc * 512:(oc + 1) * 512],
                                    start=(dt == 0), stop=(dt == 15))
                    for tt in range(4):
                        for oc in range(2):
                            oca = grp * 2 + oc
                            xr = scrR.tile([128, TPC], f32, tag="xres",
                                           name="xres")
                            nc.sync.dma_start(
                                out=xr, in_=x_d[tt * 128:(tt + 1) * 128,
                                                oca * 512:(oca + 1) * 512])
                            nc.vector.scalar_tensor_tensor(
                                out=x2[tt][:, oca * 512:(oca + 1) * 512],
                                in0=pss[tt * 2 + oc][:], scalar=scaleB[tt],
                                in1=xr, op0=Alu.mult, op1=Alu.add)

            # =========================================================
            # P4: FFN
            # =========================================================
            with ExitStack() as s5:
                up = s5.enter_context(tc.tile_pool(name="up", bufs=1))
                xqcp = s5.enter_context(tc.tile_pool(name="xqcp", bufs=1))
                uq2 = s5.enter_context(tc.tile_pool(name="uq2", bufs=6))

                xqc = [xqcp.tile([128, D], bf16, name=f"xqc{i}")
                       for i in range(4)]
                for tt in range(4):
                    hs = scrA.tile([128, D], f32, tag="work", name="work")
                    ln(x2[tt], hs)
                    quant(hs, xqc[tt], scaleC[tt])
                for dt in range(16):
                    for tt in range(4):
                        nc.scalar.dma_start(
                            out=xqt[dt][:, tt * 128:(tt + 1) * 128],
                            in_=xqc[tt][:, dt * 128:(dt + 1) * 128],
                            transpose=True)

                # FFN up, token-major U kept in SBUF
                U = [up.tile([128, F], bf16, name=f"U{i}")
                     for i in range(4)]
                with ExitStack() as s5b:
                    wsP4u = s5b.enter_context(tc.tile_pool(name="wsP4u",
                                                           bufs=4))
                    for fc in range(16):
                        pss = [psum.tile([128, TPC], f32, tag="ps",
                                         name="ps") for _ in range(4)]
                        for dt in range(16):
                            wt = wsP4u.tile([128, 512], bf16, tag="wstu",
                                            name="wstu")
                            nc.sync.dma_start(out=wt,
                                              in_=wT("wu", dt, fc * 512,
                                                     (fc + 1) * 512))
                            for tt in range(4):
                                nc.tensor.matmul(
                                    pss[tt][:],
                                    xqt[dt][:, tt * 128:(tt + 1) * 128],
                                    wt[:], start=(dt == 0),
                                    stop=(dt == 15))
                        for tt in range(4):
                            nc.scalar.activation(
                                out=U[tt][:, fc * 512:(fc + 1) * 512],
                                in_=pss[tt][:], func=Act.Gelu,
                                scale=scaleC[tt])
                            nc.vector.tensor_reduce(
                                out=amaxD[tt][:, fc:fc + 1],
                                in_=U[tt][:, fc * 512:(fc + 1) * 512],
                                axis=Ax.X, op=Alu.max,
                                apply_absolute_value=True)
                rqD = [p0.tile([128, 1], f32, name=f"rqD{i}")
                       for i in range(4)]
                for tt in range(4):
                    am = stats.tile([128, 1], f32, tag="amax", name="amax")
                    nc.vector.tensor_reduce(out=am, in_=amaxD[tt],
                                            axis=Ax.X, op=Alu.max)
                    r = quant_rq(am, scaleD[tt])
                    nc.vector.tensor_copy(out=rqD[tt], in_=r)
                # quantize U in place (integers in bf16)
                for tt in range(4):
                    for qtr in range(4):
                        sl = U[tt][:, qtr * 2048:(qtr + 1) * 2048]
                        qf = scrA.tile([128, 2048], f32, tag="work",
                                       name="work")
                        VE().tensor_scalar(out=qf, in0=sl,
                                           scalar1=rqD[tt], scalar2=MAGIC,
                                           op0=Alu.mult, op1=Alu.add)
                        VE().tensor_scalar(out=sl, in0=qf, scalar1=MAGIC,
                                           scalar2=None, op0=Alu.subtract)

                # FFN down: stream wd, transpose Uq tiles on the fly
                with ExitStack() as s5c:
                    wsP4d = s5c.enter_context(tc.tile_pool(name="wsP4d",
                                                           bufs=3))
                    for tp_ in range(2):
                        tts = (tp_ * 2, tp_ * 2 + 1)
                        pss = [psum.tile([128, TPC], f32, tag="ps",
                                         name="ps") for _ in range(5)]
                        pss += [psum.tile([128, TPC], f32, tag="oT",
                                          name="psx", bufs=2)
                                for _ in range(2)]
                        pss += [psum.tile([128, TPC], f32, tag="se",
                                          name="psy", bufs=1)]
                        for dt in range(64):
                            xdt = uq2.tile([128, 256], bf16, tag="xdt",
                                           name="xdt")
                            for ti, tt in enumerate(tts):
                                nc.scalar.dma_start(
                                    out=xdt[:, ti * 128:(ti + 1) * 128],
                                    in_=U[tt][:, dt * 128:(dt + 1) * 128],
                                    transpose=True)
                            wt = wsP4d.tile([128, 2048], bf16, tag="wstd",
                                            name="wstd")
                            nc.sync.dma_start(out=wt,
                                              in_=wT("wd", dt, 0, D))
                            for ti in range(2):
                                for oc in range(4):
                                    nc.tensor.matmul(
                                        pss[ti * 4 + oc][:],
                                        xdt[:, ti * 128:(ti + 1) * 128],
                                        wt[:, oc * 512:(oc + 1) * 512],
                                        start=(dt == 0), stop=(dt == 63))
                        for ti, tt in enumerate(tts):
                            for oc in range(4):
                                ot = scrR.tile([128, TPC], f32, tag="ot",
                                               name="ot")
                                nc.vector.scalar_tensor_tensor(
                                    out=ot, in0=pss[ti * 4 + oc][:],
                                    scalar=scaleD[tt],
                                    in1=x2[tt][:, oc * 512:(oc + 1) * 512],
                                    op0=Alu.mult, op1=Alu.add)
                                nc.gpsimd.dma_start(
                                    out=out_d[tt * 128:(tt + 1) * 128,
                                              oc * 512:(oc + 1) * 512],
                                    in_=ot)

    nc.finalize()
    return nc


_CACHE = {}


def kernel(**inputs):
    x = np.asarray(inputs["x"], dtype=np.float32)
    wq = np.asarray(inputs["wq"], dtype=np.float32)
    wk = np.asarray(inputs["wk"], dtype=np.float32)
    wv = np.asarray(inputs["wv"], dtype=np.float32)
    wo = np.asarray(inputs["wo"], dtype=np.float32)
    wu = np.asarray(inputs["wu"], dtype=np.float32)
    wd = np.asarray(inputs["wd"], dtype=np.float32)

    if "nc" not in _CACHE:
        _CACHE["nc"] = build_graph()
    nc = _CACHE["nc"]

    in_maps = []
    for r in range(NCORES):
        b, c = r // 4, r % 4
        blks = (c, 7 - c)
        xr = np.concatenate(
            [x[b, blk * 256:(blk + 1) * 256, :] for blk in blks], axis=0)
        # transposed additive causal mask: [kt, kv_local, q_local]
        maskT = np.zeros((16, 128, 256), dtype=np.float32)
        kk = np.arange(128)[:, None]
        qq = np.arange(256)[None, :]
        for kt in range(16):
            kv_abs = kt * 128 + kk
            blk = blks[0] if kt < 8 else blks[1]
            q_abs = blk * 256 + qq
            maskT[kt] = np.where(kv_abs <= q_abs, 0.0, NEG)
        cs = slice(r * 256, (r + 1) * 256)
        in_maps.append({
            "x": np.ascontiguousarray(xr),
            "wq_sl": np.ascontiguousarray(wq[:, cs].T),
            "wk_sl": np.ascontiguousarray(wk[:, cs].T),
            "wv_sl": np.ascontiguousarray(wv[:, cs].T),
            "wo_sl": np.ascontiguousarray(wo[:, cs].T),
            "wu_sl": np.ascontiguousarray(wu[:, cs].T),
            "wd_sl": np.ascontiguousarray(
                wd[:, r * 1024:(r + 1) * 1024].T),
            "maskT": maskT,
        })

    from concourse import bass_utils
    res = bass_utils.run_bass_kernel_spmd(
        nc, in_maps, core_ids=list(range(NCORES)))
    _CACHE["last_result"] = res

    out = np.zeros((B, T, D), dtype=np.float32)
    for r in range(NCORES):
        b, c = r // 4, r % 4
        o = res.results[r]["out"]
        out[b, c * 256:(c + 1) * 256, :] = o[0:256]
        out[b, (7 - c) * 256:(8 - c) * 256, :] = o[256:512]
    return out


if __name__ == "__main__":
    nc = build_graph()
    n_inst = sum(len(bb.instructions) for bb in nc.main_func.blocks)
    print("graph built ok, instructions:", n_inst)
